# revision 51
# baseline (speedup 1.0000x reference)
"""Distributed multi-head attention kernel for 8 TRN2 NeuronCores.

Problem: x[2,2048,1024] -> qkv proj -> per-head RoPE (indexed by HEAD, a
fixed linear transform) -> attention (16 heads, d=64) -> out proj.

Sharding: core c handles batch c//4 and heads 4*(c%4) .. 4*(c%4)+3.
After attention, a per-chunk 4-rank AllToAll exchanges bf16 attention
outputs so each core holds ALL 16 heads for its own 128-row q-slice of
the chunk; the out projection then runs locally with the full out_w
(K=1024 accumulated in PSUM) -- no reduction collective anywhere, and
the host only concatenates row blocks.

Host-side folds: RoPE rotation and the 1/sqrt(64) score scale are folded
into qkv_w columns; matmul inputs are pre-rounded to tf32 (float32r),
which streams at full PE rate. P = exp(S) and V are bf16 (same PE rate,
half the SBUF); softmax denominators come from a ones-column in V_aug.

Schedule: pass 1 projects K^T/V^T for the whole sequence (x^T streamed);
V^T is PE-transposed to V. Pass 2 projects Q^T one 512-wide chunk at a
time and immediately runs that chunk's attention: S^T matmuls -> exp
direct PSUM->SBUF on ACT -> P^T @ V_aug (accumulating softmax Z in row
64) -> reciprocal * broadcast -> AllToAll -> out-projection. Engines
pipeline across chunks.
"""
import sys
for _p in ("/opt/trn_rl_repo", "/root/.axon_site/_ro/trn_rl_repo"):
    if _p not in sys.path:
        sys.path.insert(0, _p)

import numpy as np

from concourse import bacc, tile, bass_utils
from concourse import mybir

F32 = mybir.dt.float32
F32R = mybir.dt.float32r
BF16 = mybir.dt.bfloat16
F8 = mybir.dt.float8e4
DR = mybir.MatmulPerfMode.DoubleRow
W8 = 32.0           # fp8 weight pre-scale (host); undone via exp scale/vones
EXP = mybir.ActivationFunctionType.Exp

HID = 1024
SEQ = 2048
HEADS = 16
D = 64
HPC = 4            # heads per core
N_CORES = 8
QC = 512           # q-chunk (free dim of scores matmuls)
NQ = SEQ // QC     # 4 q-chunks
KT = SEQ // 128    # 16 key tiles
VW = D + 1         # v_aug width per head (ones column at 64)


def _round_tf32(x):
    u = np.ascontiguousarray(x, dtype=np.float32).view(np.uint32).copy()
    u += 0xFFF + ((u >> 13) & 1)
    u &= np.uint32(0xFFFFE000)
    return u.view(np.float32)


def _rope_mats():
    """M_h [64,64] per head h: q_rot = q @ M_h (head-indexed RoPE quirk)."""
    j = np.arange(0, D, 2, dtype=np.float64) / D
    inv_freq = 1.0 / (10000.0 ** j)              # [32]
    h = np.arange(HEADS, dtype=np.float64)
    freqs = h[:, None] * inv_freq[None, :]       # [16, 32]
    cos = np.cos(freqs).astype(np.float32)
    sin = np.sin(freqs).astype(np.float32)
    mats = np.zeros((HEADS, D, D), np.float32)
    idx = np.arange(D // 2)
    for hh in range(HEADS):
        mats[hh, idx, idx] = cos[hh]
        mats[hh, D // 2 + idx, idx] = -sin[hh]
        mats[hh, idx, D // 2 + idx] = sin[hh]
        mats[hh, D // 2 + idx, D // 2 + idx] = cos[hh]
    return mats


_NC_CACHE = {}


def _build(with_collectives=True, n_cores=N_CORES):
    key = (with_collectives, n_cores)
    if key in _NC_CACHE:
        return _NC_CACHE[key]
    nc = bacc.Bacc("TRN2", target_bir_lowering=False, debug=False,
                   num_devices=n_cores)

    # weight column tiles ct: 0=q01 1=q23 2=k01 3=k23 4=v01 5=v23
    xt = nc.dram_tensor("xt", [HID, SEQ], BF16, kind="ExternalInput")
    wall = nc.dram_tensor("wall", [HID, 12 * D], BF16, kind="ExternalInput")
    w2 = nc.dram_tensor("w2", [HID, HID], BF16, kind="ExternalInput")
    ball = nc.dram_tensor("ball", [128, 6], F32, kind="ExternalInput")
    bo = nc.dram_tensor("bo", [1, HID], F32, kind="ExternalInput")
    ones_i = nc.dram_tensor("ones_i", [1, 128], F32, kind="ExternalInput")
    ident = nc.dram_tensor("ident", [128, 128], BF16, kind="ExternalInput")
    vones = nc.dram_tensor("vones", [128, KT * HPC], BF16, kind="ExternalInput")
    out_e = nc.dram_tensor("out", [QC, HID], F32, kind="ExternalOutput")

    with tile.TileContext(nc) as tc:
        with tc.tile_pool(name="const", bufs=1) as cpool, \
             tc.tile_pool(name="work", bufs=1) as wpool, \
             tc.tile_pool(name="xts", bufs=1) as xpool, \
             tc.tile_pool(name="psum", bufs=1, space="PSUM") as pp, \
             tc.tile_pool(name="dram", bufs=1, space="DRAM") as dpool:

            # ---- constant loads
            wall_sb = cpool.tile([128, 8 * 768], BF16)     # k-tile k at [:, 768k:+768]
            w2_sb = cpool.tile([128, 8 * HID], BF16)       # full out_w, row-block kb at [:, kb*HID:+HID]
            ball_sb = cpool.tile([128, 6], F32)
            bo_sb = cpool.tile([1, HID], F32)
            ones_sb = cpool.tile([1, 128], F32)
            id_sb = cpool.tile([128, 128], BF16)
            nc.gpsimd.dma_start(id_sb[:], ident.ap()[:])
            nc.gpsimd.dma_start(ball_sb[:], ball.ap()[:])
            bob_sb = cpool.tile([128, HID], F32)

            # ---- persistent activations
            qkT_sb = wpool.tile([128, 4 * SEQ], BF16)   # col-tile ct at [:, ct*SEQ:+SEQ]
            vT_sb = wpool.tile([128, 2 * SEQ], BF16)
            v_sb = wpool.tile([128, KT * HPC * VW], BF16)
            outT_sb = wpool.tile([128, 2 * SEQ], BF16)

            def xt_dma(nq, k, eng):
                t = xpool.tile([128, 512], BF16, tag="xts", bufs=10,
                               name=f"xt_{nq}_{k}")
                eng.dma_start(t[:], xt.ap()[128 * k:128 * (k + 1),
                                            QC * nq:QC * (nq + 1)])
                return t

            CH0 = (0, 0, 512)
            CH1 = (1, 512, 512)
            # P blocks filled during pass 1: all of q-chunk 0 (one tile per
            # head pair) plus chunk 1 hp0's first 4 groups; key tile kt's
            # [halfA | halfB] block lives at [:, 1024*kt:+1024]
            pt0 = [wpool.tile([128, 2 * KT * QC], BF16, tag="pt", bufs=3,
                              name=f"pt0_{i}") for i in range(2)]
            pt1h0 = wpool.tile([128, 2 * KT * QC], BF16, tag="pt", bufs=3,
                               name="pt1h0")
            PRE_KG = 4         # chunk-1 hp0 groups prefetched in pass 1

            def scores_exp(ch, hp, kg, pt):
                """S^T matmuls for head pair hp of one exp-group: 2 key
                tiles of q-chunk ch; exp directly PSUM -> SBUF (bf16).
                Both 64-row halves write ONE ps tile so they become ready
                together and overlap in the two array halves."""
                if pt is None:
                    return
                _, q_off, q_len = ch
                assert q_len == QC
                qslc = slice(SEQ * hp + q_off, SEQ * hp + q_off + q_len)
                for j in range(2):
                    kt = 2 * kg + j
                    kslc = slice(SEQ * (2 + hp) + 128 * kt,
                                 SEQ * (2 + hp) + 128 * (kt + 1))
                    ps = pp.tile([128, 1024], F32, tag="s", bufs=2,
                                 name=f"ps_{q_off}_{hp}_{kt}")
                    for base in (0, 64):
                        nc.tensor.matmul(ps[:, 8 * base:8 * base + 512],
                                         lhsT=qkT_sb[base:base + 64, kslc],
                                         rhs=qkT_sb[base:base + 64, qslc],
                                         start=True, stop=True,
                                         tile_position=(base, 0))
                    nc.scalar.activation(pt[:, 1024 * kt:1024 * (kt + 1)],
                                         ps[:], EXP)

            def v_mm(oacc, h, kt, pt, half, start, stop):
                nc.tensor.matmul(
                    oacc[:, :QC],
                    lhsT=v_sb[:, VW * HPC * kt + VW * h:VW * HPC * kt + VW * (h + 1)],
                    rhs=pt[:, 1024 * kt + 512 * half:1024 * kt + 512 * (half + 1)],
                    start=start, stop=stop)

            def normalize(ch, hp, half, oacc):
                _, q_off, q_len = ch
                h = 2 * hp + half
                # stage Z at partition 0: reciprocal_approx_fast reads
                # partition-offset inputs incorrectly (measured), and ~5x
                # faster than exact reciprocal otherwise
                zs = wpool.tile([1, 512], F32, tag="zs", bufs=2,
                                name=f"zs_{q_off}_{h}")
                nc.vector.tensor_copy(zs[:, :q_len], oacc[D:D + 1, :q_len])
                rz = wpool.tile([1, 512], F32, tag="rz", bufs=2,
                                name=f"rz_{q_off}_{h}")
                with nc.allow_low_precision(reason="recip feeds bcast matmul"):
                    nc.vector.reciprocal_approx_fast(rz[:, :q_len],
                                                     zs[:, :q_len])
                # copy the unnormalized head output out of PSUM while the
                # reciprocal/broadcast run; the multiply then reads the
                # broadcast straight from PSUM (one DVE hop shorter)
                ou = wpool.tile([64, 512], F32, tag="bc", bufs=2, name=f"ou_{q_off}_{h}")
                nc.vector.tensor_copy(ou[:, :q_len], oacc[0:D, :q_len])
                bcm = pp.tile([64, 512], F32, tag="pr", bufs=2, name=f"bcm_{q_off}_{h}")
                nc.tensor.matmul(bcm[:, :q_len], lhsT=ones_sb[:, :64],
                                 rhs=rz[:, :q_len], start=True, stop=True)
                nc.vector.tensor_tensor(
                    outT_sb[64 * half:64 * (half + 1),
                            SEQ * hp + q_off:SEQ * hp + q_off + q_len],
                    bcm[:, :q_len], ou[:, :q_len],
                    mybir.AluOpType.mult)

            # ~5us of dummy matmuls at t=0: releases the HAM clock gate
            # before the real stream starts (the PE otherwise idles waiting
            # for the first x tiles and begins pass 1 cold at 1.2 GHz)
            spin = pp.tile([128, 128], F32, tag="pr", bufs=2, name="spin")
            for _ in range(24):
                nc.tensor.matmul(spin[:], lhsT=id_sb[:], rhs=id_sb[:],
                                 start=True, stop=True)

            # ---- pass 1: project K^T, then Q^T, then V^T chunk by chunk
            # (3 sweeps over resident x^T tiles); early scores+exp for
            # q-chunk 0 keep ACT busy while the PE projects.
            for nq in range(NQ):
                if nq == 1 and with_collectives:
                    # warm-up AllToAll (16KB of identity-matrix bytes):
                    # absorbs the collective stream's first-op latency (and
                    # the start barrier) mid pass 1 so chunk 0's real
                    # exchange starts promptly.
                    warm_in = dpool.tile([8 * 128, 8], BF16, name="warm_in")
                    warm_out = dpool.tile([8 * 128, 8], BF16, name="warm_out")
                    for a in range(8):
                        nc.gpsimd.dma_start(warm_in[128 * a:128 * (a + 1), :],
                                            id_sb[:, 0:8])
                    nc.gpsimd.collective_compute(
                        "AllToAll",
                        mybir.AluOpType.bypass,
                        replica_groups=[list(range(8))],
                        ins=[warm_in[:].opt()],
                        outs=[warm_out[:].opt()],
                    )
                # per-column-tile 1-bank accumulators (tags oacc/pr) keep
                # the whole "s" tag free for score tiles -- pass-1 proj no
                # longer serializes against the early-chunk exps
                def proj_pair(cts, xts_seq, tag_pair=("oacc", "pr")):
                    accs = [pp.tile([128, 512], F32, tag=tg, bufs=2,
                                    name=f"acc{ct}_{nq}")
                            for ct, tg in zip(cts, tag_pair)]
                    for k, xt_t in xts_seq:
                        for acc, ct in zip(accs, cts):
                            nc.tensor.matmul(
                                acc[:],
                                lhsT=wall_sb[:, 768 * k + 128 * ct:768 * k + 128 * (ct + 1)],
                                rhs=xt_t[:], start=(k == 0), stop=(k == 7))
                    for acc, ct in zip(accs, cts):
                        nc.vector.tensor_scalar_add(
                            qkT_sb[:, SEQ * ct + QC * nq:SEQ * ct + QC * (nq + 1)],
                            acc[:], ball_sb[:, ct:ct + 1])

                xts = []

                def xts_stream():
                    for k in range(8):
                        if nq == 0:
                            nc.gpsimd.dma_start(
                                wall_sb[:, 768 * k:768 * (k + 1)],
                                wall.ap()[128 * k:128 * (k + 1), :])
                        # spread the x^T stream over 3 DMA queues (gpsimd
                        # is loading weights during nq 0)
                        if nq == 0:
                            xt_eng = nc.sync if k % 2 == 0 else nc.scalar
                        else:
                            xt_eng = (nc.sync, nc.scalar, nc.gpsimd)[k % 3]
                        xt_t = xt_dma(nq, k, xt_eng)
                        xts.append(xt_t)
                        yield k, xt_t

                if nq == 0:
                    # chunk 0: K and Q interleaved so the first scores fire
                    # asap (4 live accumulator banks)
                    stream = xts_stream()
                    accs = [pp.tile([128, 512], F32, tag=tg, bufs=2,
                                    name=f"acc{ct}_0")
                            for ct, tg in zip((2, 3, 0, 1),
                                              ("oacc", "pr", "oacc", "pr"))]
                    for k, xt_t in stream:
                        for acc, ct in zip(accs, (2, 3, 0, 1)):
                            nc.tensor.matmul(
                                acc[:],
                                lhsT=wall_sb[:, 768 * k + 128 * ct:768 * k + 128 * (ct + 1)],
                                rhs=xt_t[:], start=(k == 0), stop=(k == 7))
                    for acc, ct in zip(accs, (2, 3, 0, 1)):
                        nc.vector.tensor_scalar_add(
                            qkT_sb[:, SEQ * ct + QC * nq:SEQ * ct + QC * (nq + 1)],
                            acc[:], ball_sb[:, ct:ct + 1])
                    scores_exp(CH0, 0, 0, pt0[0])
                else:
                    proj_pair((2, 3), xts_stream())
                    scores_exp(CH0, 0, 2 * nq, pt0[0])
                    proj_pair((0, 1), ((k, xts[k]) for k in range(8)))
                scores_exp(CH0, 1, 2 * nq, pt0[1])
                scores_exp(CH0, 0, 2 * nq + 1, pt0[0])
                if nq >= 2:
                    # chunk-1 hp0 prefetch: Q(ch1) ready after nq=1, K key
                    # tiles 0..7 after nq=1 -- steal idle ACT time here
                    scores_exp(CH1, 0, 2 * (nq - 2), pt1h0)
                    scores_exp(CH1, 0, 2 * (nq - 2) + 1, pt1h0)
                vA = pp.tile([128, 512], F32, tag="oacc", bufs=2, name=f"vA_{nq}")
                vB = pp.tile([128, 512], F32, tag="pr", bufs=2, name=f"vB_{nq}")
                for k in range(8):
                    nc.tensor.matmul(
                        vA[:], lhsT=wall_sb[:, 768 * k + 512:768 * k + 640],
                        rhs=xts[k][:], start=(k == 0), stop=(k == 7))
                    nc.tensor.matmul(
                        vB[:], lhsT=wall_sb[:, 768 * k + 640:768 * k + 768],
                        rhs=xts[k][:], start=(k == 0), stop=(k == 7))
                nc.vector.tensor_scalar_add(
                    vT_sb[:, QC * nq:QC * (nq + 1)], vA[:], ball_sb[:, 4:5])
                nc.vector.tensor_scalar_add(
                    vT_sb[:, SEQ + QC * nq:SEQ + QC * (nq + 1)], vB[:], ball_sb[:, 5:6])
                # V^T -> V (natural, bf16) for this quarter of the keys
                for cv in range(2):
                    for st in range(4 * nq, 4 * nq + 4):
                        tp = pp.tile([128, 128], BF16, tag="pr", bufs=2,
                                     name=f"tp_{cv}_{st}")
                        nc.tensor.transpose(
                            tp[:], vT_sb[:, SEQ * cv + 128 * st:SEQ * cv + 128 * (st + 1)],
                            id_sb[:])
                        dst = v_sb[:, VW * HPC * st + 2 * VW * cv:VW * HPC * st + 2 * VW * (cv + 1)]
                        nc.vector.tensor_copy(
                            dst.rearrange("p (h w) -> p h w", h=2, w=VW)[:, :, :D],
                            tp[:].rearrange("p (h w) -> p h w", h=2, w=D),
                        )
                scores_exp(CH0, 1, 2 * nq + 1, pt0[1])

            # ones columns of v_aug: one strided DMA ([128, 64] with free stride VW)
            nc.sync.dma_start(
                v_sb[:].rearrange("p (i w) -> p i w", i=KT * HPC, w=VW)[:, :, D],
                vones.ap()[:],
            )
            # ---- deferred constants (needed from pass 2 on)
            for k in range(8):
                nc.sync.dma_start(w2_sb[:, HID * k:HID * (k + 1)],
                                  w2.ap()[128 * k:128 * (k + 1), :])
            nc.sync.dma_start(bo_sb[:], bo.ap()[:])
            nc.sync.dma_start(ones_sb[:], ones_i.ap()[:])
            # out-bias broadcast ([1,N] -> [128,N] via K=1 ones matmul)
            for nn in range(2):
                ps_bo = pp.tile([128, 512], F32, tag="pr", bufs=2)
                nc.tensor.matmul(ps_bo[:], lhsT=ones_sb[:, :128],
                                 rhs=bo_sb[:, 512 * nn:512 * (nn + 1)],
                                 start=True, stop=True)
                nc.vector.tensor_copy(bob_sb[:, 512 * nn:512 * (nn + 1)], ps_bo[:])

            # q-chunks: (index, q_off, q_len)
            CHUNKS = [(0, 0, 512), (1, 512, 512), (2, 1024, 512),
                      (3, 1536, 512)]
            # per-chunk output row offset in out_e
            OUT_ROW = {0: 0, 1: 128, 2: 256, 3: 384}
            QPB = 64           # q rows per batch per core per chunk

            # ---- pass 2 out projection, 8-rank AllToAll per (chunk, head
            # pair): dst core c' gets this core's heads 2hp,2hp+1 for q rows
            # [q_off+64c' .. +64); afterwards each core holds ALL 16 heads of
            # BOTH batches for its own 64-row q-slice and runs the out
            # projection locally (batches packed side by side, K=1024).
            # hp=0's exchange launches mid-chunk, hp=1's right after the
            # chunk; the PE-consuming fin runs deep inside the next chunk.
            a2a_outs = {}

            def out_send(ch, hp):
                idx, q_off, q_len = ch
                a2a_in = dpool.tile([8 * 128, QPB], BF16, tag="a2ain",
                                    bufs=6, name=f"a2ain_{idx}_{hp}")
                for cd in range(8):
                    nc.sync.dma_start(
                        a2a_in[128 * cd:128 * (cd + 1), :],
                        outT_sb[:, SEQ * hp + q_off + QPB * cd:
                                SEQ * hp + q_off + QPB * (cd + 1)])
                a2a_out = dpool.tile([8 * 128, QPB], BF16, tag="a2aout",
                                     bufs=6, name=f"a2aout_{idx}_{hp}")
                a2a_outs[(idx, hp)] = a2a_out
                if with_collectives:
                    nc.gpsimd.collective_compute(
                        "AllToAll",
                        mybir.AluOpType.bypass,
                        replica_groups=[list(range(8))],
                        ins=[a2a_in[:].opt()],
                        outs=[a2a_out[:].opt()],
                    )
                else:
                    nc.sync.dma_start(a2a_out[:], a2a_in[:])

            def out_fin(ch):
                idx, q_off, q_len = ch
                # agg block kb=2*rs+hp holds out_w rows 128kb..128(kb+1) as
                # partitions; cols 0:64 = batch 0 (core rs), 64:128 = batch 1
                # (core rs+4), q rows are this core's own slice.
                agg = wpool.tile([128, 8 * 128], BF16, tag="agg", bufs=2,
                                 name=f"agg_{idx}")
                for hp in range(2):
                    a2a_out = a2a_outs.pop((idx, hp))
                    for s in range(8):
                        rs, gs = s % 4, s // 4
                        kb = 2 * rs + hp
                        nc.gpsimd.dma_start(
                            agg[:, 128 * kb + QPB * gs:128 * kb + QPB * (gs + 1)],
                            a2a_out[128 * s:128 * (s + 1), :])
                ob = wpool.tile([128, HID], F32, tag="ob", bufs=2,
                                name=f"ob_{idx}")
                psos = [pp.tile([128, 512], F32, tag="pr", bufs=2,
                                name=f"pso_{idx}_{nn}") for nn in range(2)]
                for kb in range(8):
                    for nn in range(2):
                        nc.tensor.matmul(
                            psos[nn][:],
                            lhsT=agg[:, 128 * kb:128 * (kb + 1)],
                            rhs=w2_sb[:, HID * kb + 512 * nn:HID * kb + 512 * (nn + 1)],
                            start=(kb == 0), stop=(kb == 7))
                for nn in range(2):
                    nc.vector.tensor_tensor(
                        ob[:, 512 * nn:512 * (nn + 1)], psos[nn][:],
                        bob_sb[:, 512 * nn:512 * (nn + 1)],
                        mybir.AluOpType.add)
                nc.sync.dma_start(
                    out_e.ap()[OUT_ROW[idx]:OUT_ROW[idx] + 128, :], ob[:, :HID])

            sent = []          # chunks whose hp=1 exchange is in flight
            pending = None
            pending_norm = []
            for ch in CHUNKS:
                idx, q_off, q_len = ch
                G = 1024 // q_len
                for hp in range(2):
                    pre = {(0, 0): (pt0[0], 8), (0, 1): (pt0[1], 8),
                           (1, 0): (pt1h0, PRE_KG)}.get((idx, hp))
                    ptAB, kg_pre = pre if pre else (
                        wpool.tile([128, 2 * KT * q_len], BF16, tag="pt",
                                   bufs=3, name=f"pt_{idx}_{hp}"), 0)
                    oaccs = [pp.tile([VW, 512], F32, tag="oacc", bufs=2,
                                     name=f"oacc_{idx}_{2 * hp + half}")
                             for half in range(2)]
                    def v_group(kg):
                        for half in range(2):
                            for j in range(G):
                                kt = G * kg + j
                                v_mm(oaccs[half], 2 * hp + half, kt, ptAB,
                                     half, kt == 0, kt == KT - 1)
                    # V matmuls trail the scores/exp by one group so the PE
                    # never head-of-line blocks on the exp it just requested
                    for kg in range(KT // G):
                        if kg >= kg_pre:
                            scores_exp(ch, hp, kg, ptAB)
                        if kg == 0:
                            for args in pending_norm:
                                normalize(*args)
                            pending_norm = []
                        else:
                            v_group(kg - 1)
                        if kg == 1:
                            if hp == 0 and pending is not None:
                                # previous chunk: ship its hp=1 heads
                                out_send(pending, 1)
                                sent.append(pending)
                                pending = None
                            elif hp == 1:
                                # this chunk's hp=0 heads are normalized by
                                # now -- ship them early (trigger before
                                # fin's readbacks claim the gpsimd queue)
                                out_send(ch, 0)
                                # project a chunk whose exchange has had a
                                # full chunk of slack (absorbs launch skew
                                # across cores)
                                if len(sent) >= 2:
                                    out_fin(sent.pop(0))
                    v_group(KT // G - 1)
                    for half in range(2):
                        pending_norm.append((ch, hp, half, oaccs[half]))
                pending = ch
            for args in pending_norm:
                normalize(*args)
            out_send(pending, 1)
            sent.append(pending)
            for ch in sent:
                out_fin(ch)

    nc.compile()
    _NC_CACHE[key] = nc
    return nc


def _prep_in_maps(x, qkv_w, qkv_b, out_w, out_b):
    """Per-core input tensors; w2/bo are now the FULL out_w/out_b."""
    mats = _rope_mats()
    x = np.asarray(x, np.float32)
    qkv_w = np.asarray(qkv_w, np.float32)
    qkv_b = np.asarray(qkv_b, np.float32)
    out_w = np.asarray(out_w, np.float32)
    out_b = np.asarray(out_b, np.float32)

    # per-head slices of interleaved qkv (head h owns cols 192h .. 192h+192)
    wq = np.stack([qkv_w[:, 192 * h:192 * h + 64] for h in range(HEADS)])      # [16,1024,64]
    wk = np.stack([qkv_w[:, 192 * h + 64:192 * h + 128] for h in range(HEADS)])
    wv = np.stack([qkv_w[:, 192 * h + 128:192 * h + 192] for h in range(HEADS)])
    bq = np.stack([qkv_b[192 * h:192 * h + 64] for h in range(HEADS)])
    bk = np.stack([qkv_b[192 * h + 64:192 * h + 128] for h in range(HEADS)])
    bvv = np.stack([qkv_b[192 * h + 128:192 * h + 192] for h in range(HEADS)])

    import ml_dtypes
    scale = 1.0 / np.sqrt(D)
    wq_r = np.einsum("hij,hjk->hik", wq, mats) * scale
    bq_r = np.einsum("hj,hjk->hk", bq, mats) * scale
    wk_r = np.einsum("hij,hjk->hik", wk, mats)
    bk_r = np.einsum("hj,hjk->hk", bk, mats)

    in_maps = []
    for c in range(N_CORES):
        g, r = divmod(c, 4)
        hs = [4 * r + i for i in range(HPC)]
        xt = x[g].T.astype(ml_dtypes.bfloat16)                              # [1024, 2048]
        wall_c = np.concatenate([wq_r[h] for h in hs] + [wk_r[h] for h in hs]
                                + [wv[h] for h in hs], axis=1)              # [1024, 768]
        w2_c = out_w                                                        # [1024, 1024]
        ball_c = np.concatenate([bq_r[h] for h in hs] + [bk_r[h] for h in hs]
                                + [bvv[h] for h in hs])                     # [768]
        bo_c = out_b[None, :]
        in_maps.append({
            "xt": xt,
            "wall": wall_c.astype(ml_dtypes.bfloat16),
            "w2": w2_c.astype(ml_dtypes.bfloat16),
            "ball": ball_c.reshape(6, 128).T.copy().astype(np.float32),
            "bo": bo_c.astype(np.float32),
            "ones_i": np.ones((1, 128), np.float32),
            "ident": np.eye(128, dtype=ml_dtypes.bfloat16),
            "vones": np.ones((128, KT * HPC), ml_dtypes.bfloat16),
        })
    return in_maps


RUN_KWARGS = {}     # test.py sets {"trace": True} to profile; harness leaves {}
LAST_RES = None


def kernel(x, qkv_w, qkv_b, out_w, out_b):
    global LAST_RES
    in_maps = _prep_in_maps(x, qkv_w, qkv_b, out_w, out_b)
    nc = _build(with_collectives=True)
    res = None
    for attempt, backoff in enumerate((10, 20, 40, 60, 0)):
        try:
            res = bass_utils.run_bass_kernel_spmd(nc, in_maps,
                                                  core_ids=list(range(N_CORES)),
                                                  **RUN_KWARGS)
            break
        except Exception:
            if backoff == 0:
                raise
            import time as _time
            _time.sleep(backoff)
    LAST_RES = res
    out = np.empty((2, SEQ, HID), np.float32)
    for c in range(N_CORES):
        o = res.results[c]["out"]            # [512, 1024]
        # chunk j rows 128j..128j+128 = [batch0 64 | batch1 64] of q rows
        # [512j + 64c .. +64)
        for j in range(4):
            for b in range(2):
                out[b, 512 * j + 64 * c:512 * j + 64 * (c + 1)] = \
                    o[128 * j + 64 * b:128 * j + 64 * (b + 1)]
    return out



# revision 52
# speedup vs baseline: 1.0057x; 1.0057x over previous
"""Distributed multi-head attention kernel for 8 TRN2 NeuronCores.

Problem: x[2,2048,1024] -> qkv proj -> per-head RoPE (indexed by HEAD, a
fixed linear transform) -> attention (16 heads, d=64) -> out proj.

Sharding: core c handles batch c//4 and heads 4*(c%4) .. 4*(c%4)+3.
After attention, a per-chunk 4-rank AllToAll exchanges bf16 attention
outputs so each core holds ALL 16 heads for its own 128-row q-slice of
the chunk; the out projection then runs locally with the full out_w
(K=1024 accumulated in PSUM) -- no reduction collective anywhere, and
the host only concatenates row blocks.

Host-side folds: RoPE rotation and the 1/sqrt(64) score scale are folded
into qkv_w columns; matmul inputs are pre-rounded to tf32 (float32r),
which streams at full PE rate. P = exp(S) and V are bf16 (same PE rate,
half the SBUF); softmax denominators come from a ones-column in V_aug.

Schedule: pass 1 projects K^T/V^T for the whole sequence (x^T streamed);
V^T is PE-transposed to V. Pass 2 projects Q^T one 512-wide chunk at a
time and immediately runs that chunk's attention: S^T matmuls -> exp
direct PSUM->SBUF on ACT -> P^T @ V_aug (accumulating softmax Z in row
64) -> reciprocal * broadcast -> AllToAll -> out-projection. Engines
pipeline across chunks.
"""
import sys
for _p in ("/opt/trn_rl_repo", "/root/.axon_site/_ro/trn_rl_repo"):
    if _p not in sys.path:
        sys.path.insert(0, _p)

import numpy as np

from concourse import bacc, tile, bass_utils
from concourse import mybir

F32 = mybir.dt.float32
F32R = mybir.dt.float32r
BF16 = mybir.dt.bfloat16
F8 = mybir.dt.float8e4
DR = mybir.MatmulPerfMode.DoubleRow
W8 = 32.0           # fp8 weight pre-scale (host); undone via exp scale/vones
EXP = mybir.ActivationFunctionType.Exp

HID = 1024
SEQ = 2048
HEADS = 16
D = 64
HPC = 4            # heads per core
N_CORES = 8
QC = 512           # q-chunk (free dim of scores matmuls)
NQ = SEQ // QC     # 4 q-chunks
KT = SEQ // 128    # 16 key tiles
VW = D + 1         # v_aug width per head (ones column at 64)


def _round_tf32(x):
    u = np.ascontiguousarray(x, dtype=np.float32).view(np.uint32).copy()
    u += 0xFFF + ((u >> 13) & 1)
    u &= np.uint32(0xFFFFE000)
    return u.view(np.float32)


def _rope_mats():
    """M_h [64,64] per head h: q_rot = q @ M_h (head-indexed RoPE quirk)."""
    j = np.arange(0, D, 2, dtype=np.float64) / D
    inv_freq = 1.0 / (10000.0 ** j)              # [32]
    h = np.arange(HEADS, dtype=np.float64)
    freqs = h[:, None] * inv_freq[None, :]       # [16, 32]
    cos = np.cos(freqs).astype(np.float32)
    sin = np.sin(freqs).astype(np.float32)
    mats = np.zeros((HEADS, D, D), np.float32)
    idx = np.arange(D // 2)
    for hh in range(HEADS):
        mats[hh, idx, idx] = cos[hh]
        mats[hh, D // 2 + idx, idx] = -sin[hh]
        mats[hh, idx, D // 2 + idx] = sin[hh]
        mats[hh, D // 2 + idx, D // 2 + idx] = cos[hh]
    return mats


_NC_CACHE = {}


def _build(with_collectives=True, n_cores=N_CORES):
    key = (with_collectives, n_cores)
    if key in _NC_CACHE:
        return _NC_CACHE[key]
    nc = bacc.Bacc("TRN2", target_bir_lowering=False, debug=False,
                   num_devices=n_cores)

    # weight column tiles ct: 0=q01 1=q23 2=k01 3=k23 4=v01 5=v23
    xt = nc.dram_tensor("xt", [HID, SEQ], BF16, kind="ExternalInput")
    wall = nc.dram_tensor("wall", [HID, 12 * D], BF16, kind="ExternalInput")
    w2 = nc.dram_tensor("w2", [HID, HID], BF16, kind="ExternalInput")
    ball = nc.dram_tensor("ball", [128, 6], F32, kind="ExternalInput")
    bo = nc.dram_tensor("bo", [1, HID], F32, kind="ExternalInput")
    ones_i = nc.dram_tensor("ones_i", [1, 128], F32, kind="ExternalInput")
    ident = nc.dram_tensor("ident", [128, 128], BF16, kind="ExternalInput")
    vones = nc.dram_tensor("vones", [128, KT * HPC], BF16, kind="ExternalInput")
    out_e = nc.dram_tensor("out", [QC, HID], F32, kind="ExternalOutput")

    with tile.TileContext(nc) as tc:
        with tc.tile_pool(name="const", bufs=1) as cpool, \
             tc.tile_pool(name="work", bufs=1) as wpool, \
             tc.tile_pool(name="xts", bufs=1) as xpool, \
             tc.tile_pool(name="psum", bufs=1, space="PSUM") as pp, \
             tc.tile_pool(name="dram", bufs=1, space="DRAM") as dpool:

            # ---- constant loads
            wall_sb = cpool.tile([128, 8 * 768], BF16)     # k-tile k at [:, 768k:+768]
            w2_sb = cpool.tile([128, 8 * HID], BF16)       # full out_w, row-block kb at [:, kb*HID:+HID]
            ball_sb = cpool.tile([128, 6], F32)
            bo_sb = cpool.tile([1, HID], F32)
            ones_sb = cpool.tile([1, 128], F32)
            id_sb = cpool.tile([128, 128], BF16)
            nc.gpsimd.dma_start(id_sb[:], ident.ap()[:])
            nc.gpsimd.dma_start(ball_sb[:], ball.ap()[:])
            bob_sb = cpool.tile([128, HID], F32)

            # ---- persistent activations
            qkT_sb = wpool.tile([128, 4 * SEQ], BF16)   # col-tile ct at [:, ct*SEQ:+SEQ]
            vT_sb = wpool.tile([128, 2 * SEQ], BF16)
            v_sb = wpool.tile([128, KT * HPC * VW], BF16)
            outT_sb = wpool.tile([128, 2 * SEQ], BF16)

            def xt_dma(nq, k, eng):
                t = xpool.tile([128, 512], BF16, tag="xts", bufs=10,
                               name=f"xt_{nq}_{k}")
                eng.dma_start(t[:], xt.ap()[128 * k:128 * (k + 1),
                                            QC * nq:QC * (nq + 1)])
                return t

            CH0 = (0, 0, 512)
            CH1 = (1, 512, 512)
            # P blocks filled during pass 1: all of q-chunk 0 (one tile per
            # head pair) plus chunk 1 hp0's first 4 groups; key tile kt's
            # [halfA | halfB] block lives at [:, 1024*kt:+1024]
            pt0 = [wpool.tile([128, 2 * KT * QC], BF16, tag="pt", bufs=3,
                              name=f"pt0_{i}") for i in range(2)]
            pt1h0 = wpool.tile([128, 2 * KT * QC], BF16, tag="pt", bufs=3,
                               name="pt1h0")
            PRE_KG = 6         # chunk-1 hp0 groups prefetched in pass 1

            def scores_exp(ch, hp, kg, pt):
                """S^T matmuls for head pair hp of one exp-group: 2 key
                tiles of q-chunk ch; exp directly PSUM -> SBUF (bf16).
                Both 64-row halves write ONE ps tile so they become ready
                together and overlap in the two array halves."""
                if pt is None:
                    return
                _, q_off, q_len = ch
                assert q_len == QC
                qslc = slice(SEQ * hp + q_off, SEQ * hp + q_off + q_len)
                for j in range(2):
                    kt = 2 * kg + j
                    kslc = slice(SEQ * (2 + hp) + 128 * kt,
                                 SEQ * (2 + hp) + 128 * (kt + 1))
                    # dependency-free PE filler: bridges the short wait for
                    # the previous group's exp so the HAM activity monitor
                    # never sees an idle window and re-throttles the clock
                    for _f in range(2):
                        nc.tensor.ldweights(id_sb[:])
                    ps = pp.tile([128, 1024], F32, tag="s", bufs=2,
                                 name=f"ps_{q_off}_{hp}_{kt}")
                    for base in (0, 64):
                        nc.tensor.matmul(ps[:, 8 * base:8 * base + 512],
                                         lhsT=qkT_sb[base:base + 64, kslc],
                                         rhs=qkT_sb[base:base + 64, qslc],
                                         start=True, stop=True,
                                         tile_position=(base, 0))
                    nc.scalar.activation(pt[:, 1024 * kt:1024 * (kt + 1)],
                                         ps[:], EXP)

            def v_mm(oacc, h, kt, pt, half, start, stop):
                nc.tensor.matmul(
                    oacc[:, :QC],
                    lhsT=v_sb[:, VW * HPC * kt + VW * h:VW * HPC * kt + VW * (h + 1)],
                    rhs=pt[:, 1024 * kt + 512 * half:1024 * kt + 512 * (half + 1)],
                    start=start, stop=stop)

            def normalize(ch, hp, half, oacc):
                _, q_off, q_len = ch
                h = 2 * hp + half
                # stage Z at partition 0: reciprocal_approx_fast reads
                # partition-offset inputs incorrectly (measured), and ~5x
                # faster than exact reciprocal otherwise
                zs = wpool.tile([1, 512], F32, tag="zs", bufs=2,
                                name=f"zs_{q_off}_{h}")
                nc.vector.tensor_copy(zs[:, :q_len], oacc[D:D + 1, :q_len])
                rz = wpool.tile([1, 512], F32, tag="rz", bufs=2,
                                name=f"rz_{q_off}_{h}")
                with nc.allow_low_precision(reason="recip feeds bcast matmul"):
                    nc.vector.reciprocal_approx_fast(rz[:, :q_len],
                                                     zs[:, :q_len])
                # copy the unnormalized head output out of PSUM while the
                # reciprocal/broadcast run; the multiply then reads the
                # broadcast straight from PSUM (one DVE hop shorter)
                ou = wpool.tile([64, 512], F32, tag="bc", bufs=2, name=f"ou_{q_off}_{h}")
                nc.vector.tensor_copy(ou[:, :q_len], oacc[0:D, :q_len])
                bcm = pp.tile([64, 512], F32, tag="pr", bufs=2, name=f"bcm_{q_off}_{h}")
                nc.tensor.matmul(bcm[:, :q_len], lhsT=ones_sb[:, :64],
                                 rhs=rz[:, :q_len], start=True, stop=True)
                nc.vector.tensor_tensor(
                    outT_sb[64 * half:64 * (half + 1),
                            SEQ * hp + q_off:SEQ * hp + q_off + q_len],
                    bcm[:, :q_len], ou[:, :q_len],
                    mybir.AluOpType.mult)

            # ~5us of dummy matmuls at t=0: releases the HAM clock gate
            # before the real stream starts (the PE otherwise idles waiting
            # for the first x tiles and begins pass 1 cold at 1.2 GHz)
            spin = pp.tile([128, 128], F32, tag="pr", bufs=2, name="spin")
            for _ in range(24):
                nc.tensor.matmul(spin[:], lhsT=id_sb[:], rhs=id_sb[:],
                                 start=True, stop=True)

            # ---- pass 1: project K^T, then Q^T, then V^T chunk by chunk
            # (3 sweeps over resident x^T tiles); early scores+exp for
            # q-chunk 0 keep ACT busy while the PE projects.
            for nq in range(NQ):
                if nq == 1 and with_collectives:
                    # warm-up AllToAll (16KB of identity-matrix bytes):
                    # absorbs the collective stream's first-op latency (and
                    # the start barrier) mid pass 1 so chunk 0's real
                    # exchange starts promptly.
                    warm_in = dpool.tile([8 * 128, 8], BF16, name="warm_in")
                    warm_out = dpool.tile([8 * 128, 8], BF16, name="warm_out")
                    for a in range(8):
                        nc.gpsimd.dma_start(warm_in[128 * a:128 * (a + 1), :],
                                            id_sb[:, 0:8])
                    nc.gpsimd.collective_compute(
                        "AllToAll",
                        mybir.AluOpType.bypass,
                        replica_groups=[list(range(8))],
                        ins=[warm_in[:].opt()],
                        outs=[warm_out[:].opt()],
                    )
                # per-column-tile 1-bank accumulators (tags oacc/pr) keep
                # the whole "s" tag free for score tiles -- pass-1 proj no
                # longer serializes against the early-chunk exps
                def proj_pair(cts, xts_seq, tag_pair=("oacc", "pr")):
                    accs = [pp.tile([128, 512], F32, tag=tg, bufs=2,
                                    name=f"acc{ct}_{nq}")
                            for ct, tg in zip(cts, tag_pair)]
                    for k, xt_t in xts_seq:
                        for acc, ct in zip(accs, cts):
                            nc.tensor.matmul(
                                acc[:],
                                lhsT=wall_sb[:, 768 * k + 128 * ct:768 * k + 128 * (ct + 1)],
                                rhs=xt_t[:], start=(k == 0), stop=(k == 7))
                    for acc, ct in zip(accs, cts):
                        nc.vector.tensor_scalar_add(
                            qkT_sb[:, SEQ * ct + QC * nq:SEQ * ct + QC * (nq + 1)],
                            acc[:], ball_sb[:, ct:ct + 1])

                xts = []

                def xts_stream():
                    for k in range(8):
                        if nq == 0:
                            nc.gpsimd.dma_start(
                                wall_sb[:, 768 * k:768 * (k + 1)],
                                wall.ap()[128 * k:128 * (k + 1), :])
                        # spread the x^T stream over 3 DMA queues (gpsimd
                        # is loading weights during nq 0)
                        if nq == 0:
                            xt_eng = nc.sync if k % 2 == 0 else nc.scalar
                        else:
                            xt_eng = (nc.sync, nc.scalar, nc.gpsimd)[k % 3]
                        xt_t = xt_dma(nq, k, xt_eng)
                        xts.append(xt_t)
                        yield k, xt_t

                if nq == 0:
                    # chunk 0: K and Q interleaved so the first scores fire
                    # asap (4 live accumulator banks)
                    stream = xts_stream()
                    accs = [pp.tile([128, 512], F32, tag=tg, bufs=2,
                                    name=f"acc{ct}_0")
                            for ct, tg in zip((2, 3, 0, 1),
                                              ("oacc", "pr", "oacc", "pr"))]
                    for k, xt_t in stream:
                        for acc, ct in zip(accs, (2, 3, 0, 1)):
                            nc.tensor.matmul(
                                acc[:],
                                lhsT=wall_sb[:, 768 * k + 128 * ct:768 * k + 128 * (ct + 1)],
                                rhs=xt_t[:], start=(k == 0), stop=(k == 7))
                    for acc, ct in zip(accs, (2, 3, 0, 1)):
                        nc.vector.tensor_scalar_add(
                            qkT_sb[:, SEQ * ct + QC * nq:SEQ * ct + QC * (nq + 1)],
                            acc[:], ball_sb[:, ct:ct + 1])
                    scores_exp(CH0, 0, 0, pt0[0])
                else:
                    proj_pair((2, 3), xts_stream())
                    scores_exp(CH0, 0, 2 * nq, pt0[0])
                    proj_pair((0, 1), ((k, xts[k]) for k in range(8)))
                scores_exp(CH0, 1, 2 * nq, pt0[1])
                scores_exp(CH0, 0, 2 * nq + 1, pt0[0])
                if nq >= 2:
                    # chunk-1 hp0 prefetch: Q(ch1) ready after nq=1, K key
                    # tiles 0..11 after nq=2 -- steal idle ACT time here
                    n_pre = 2 if nq == 2 else 4
                    for kgp in range(2 * (nq - 2) + (nq - 2) * 2, 0):
                        pass
                    base_kg = 0 if nq == 2 else 2
                    for kgp in range(base_kg, base_kg + n_pre):
                        scores_exp(CH1, 0, kgp, pt1h0)
                vA = pp.tile([128, 512], F32, tag="oacc", bufs=2, name=f"vA_{nq}")
                vB = pp.tile([128, 512], F32, tag="pr", bufs=2, name=f"vB_{nq}")
                for k in range(8):
                    nc.tensor.matmul(
                        vA[:], lhsT=wall_sb[:, 768 * k + 512:768 * k + 640],
                        rhs=xts[k][:], start=(k == 0), stop=(k == 7))
                    nc.tensor.matmul(
                        vB[:], lhsT=wall_sb[:, 768 * k + 640:768 * k + 768],
                        rhs=xts[k][:], start=(k == 0), stop=(k == 7))
                nc.vector.tensor_scalar_add(
                    vT_sb[:, QC * nq:QC * (nq + 1)], vA[:], ball_sb[:, 4:5])
                nc.vector.tensor_scalar_add(
                    vT_sb[:, SEQ + QC * nq:SEQ + QC * (nq + 1)], vB[:], ball_sb[:, 5:6])
                # V^T -> V (natural, bf16) for this quarter of the keys
                for cv in range(2):
                    for st in range(4 * nq, 4 * nq + 4):
                        tp = pp.tile([128, 128], BF16, tag="pr", bufs=2,
                                     name=f"tp_{cv}_{st}")
                        nc.tensor.transpose(
                            tp[:], vT_sb[:, SEQ * cv + 128 * st:SEQ * cv + 128 * (st + 1)],
                            id_sb[:])
                        dst = v_sb[:, VW * HPC * st + 2 * VW * cv:VW * HPC * st + 2 * VW * (cv + 1)]
                        nc.vector.tensor_copy(
                            dst.rearrange("p (h w) -> p h w", h=2, w=VW)[:, :, :D],
                            tp[:].rearrange("p (h w) -> p h w", h=2, w=D),
                        )
                scores_exp(CH0, 1, 2 * nq + 1, pt0[1])

            # ones columns of v_aug: one strided DMA ([128, 64] with free stride VW)
            nc.sync.dma_start(
                v_sb[:].rearrange("p (i w) -> p i w", i=KT * HPC, w=VW)[:, :, D],
                vones.ap()[:],
            )
            # ---- deferred constants (needed from pass 2 on)
            for k in range(8):
                nc.sync.dma_start(w2_sb[:, HID * k:HID * (k + 1)],
                                  w2.ap()[128 * k:128 * (k + 1), :])
            nc.sync.dma_start(bo_sb[:], bo.ap()[:])
            nc.sync.dma_start(ones_sb[:], ones_i.ap()[:])
            # out-bias broadcast ([1,N] -> [128,N] via K=1 ones matmul)
            for nn in range(2):
                ps_bo = pp.tile([128, 512], F32, tag="pr", bufs=2)
                nc.tensor.matmul(ps_bo[:], lhsT=ones_sb[:, :128],
                                 rhs=bo_sb[:, 512 * nn:512 * (nn + 1)],
                                 start=True, stop=True)
                nc.vector.tensor_copy(bob_sb[:, 512 * nn:512 * (nn + 1)], ps_bo[:])

            # q-chunks: (index, q_off, q_len)
            CHUNKS = [(0, 0, 512), (1, 512, 512), (2, 1024, 512),
                      (3, 1536, 512)]
            # per-chunk output row offset in out_e
            OUT_ROW = {0: 0, 1: 128, 2: 256, 3: 384}
            QPB = 64           # q rows per batch per core per chunk

            # ---- pass 2 out projection, 8-rank AllToAll per (chunk, head
            # pair): dst core c' gets this core's heads 2hp,2hp+1 for q rows
            # [q_off+64c' .. +64); afterwards each core holds ALL 16 heads of
            # BOTH batches for its own 64-row q-slice and runs the out
            # projection locally (batches packed side by side, K=1024).
            # hp=0's exchange launches mid-chunk, hp=1's right after the
            # chunk; the PE-consuming fin runs deep inside the next chunk.
            a2a_outs = {}

            def out_send(ch, hp):
                idx, q_off, q_len = ch
                a2a_in = dpool.tile([8 * 128, QPB], BF16, tag="a2ain",
                                    bufs=6, name=f"a2ain_{idx}_{hp}")
                for cd in range(8):
                    nc.sync.dma_start(
                        a2a_in[128 * cd:128 * (cd + 1), :],
                        outT_sb[:, SEQ * hp + q_off + QPB * cd:
                                SEQ * hp + q_off + QPB * (cd + 1)])
                a2a_out = dpool.tile([8 * 128, QPB], BF16, tag="a2aout",
                                     bufs=6, name=f"a2aout_{idx}_{hp}")
                a2a_outs[(idx, hp)] = a2a_out
                if with_collectives:
                    nc.gpsimd.collective_compute(
                        "AllToAll",
                        mybir.AluOpType.bypass,
                        replica_groups=[list(range(8))],
                        ins=[a2a_in[:].opt()],
                        outs=[a2a_out[:].opt()],
                    )
                else:
                    nc.sync.dma_start(a2a_out[:], a2a_in[:])

            def out_fin(ch):
                idx, q_off, q_len = ch
                # agg block kb=2*rs+hp holds out_w rows 128kb..128(kb+1) as
                # partitions; cols 0:64 = batch 0 (core rs), 64:128 = batch 1
                # (core rs+4), q rows are this core's own slice.
                agg = wpool.tile([128, 8 * 128], BF16, tag="agg", bufs=2,
                                 name=f"agg_{idx}")
                for hp in range(2):
                    a2a_out = a2a_outs.pop((idx, hp))
                    for s in range(8):
                        rs, gs = s % 4, s // 4
                        kb = 2 * rs + hp
                        nc.gpsimd.dma_start(
                            agg[:, 128 * kb + QPB * gs:128 * kb + QPB * (gs + 1)],
                            a2a_out[128 * s:128 * (s + 1), :])
                ob = wpool.tile([128, HID], F32, tag="ob", bufs=2,
                                name=f"ob_{idx}")
                psos = [pp.tile([128, 512], F32, tag="pr", bufs=2,
                                name=f"pso_{idx}_{nn}") for nn in range(2)]
                for kb in range(8):
                    for nn in range(2):
                        nc.tensor.matmul(
                            psos[nn][:],
                            lhsT=agg[:, 128 * kb:128 * (kb + 1)],
                            rhs=w2_sb[:, HID * kb + 512 * nn:HID * kb + 512 * (nn + 1)],
                            start=(kb == 0), stop=(kb == 7))
                for nn in range(2):
                    nc.vector.tensor_tensor(
                        ob[:, 512 * nn:512 * (nn + 1)], psos[nn][:],
                        bob_sb[:, 512 * nn:512 * (nn + 1)],
                        mybir.AluOpType.add)
                nc.sync.dma_start(
                    out_e.ap()[OUT_ROW[idx]:OUT_ROW[idx] + 128, :], ob[:, :HID])

            sent = []          # chunks whose hp=1 exchange is in flight
            pending = None
            pending_norm = []
            for ch in CHUNKS:
                idx, q_off, q_len = ch
                G = 1024 // q_len
                for hp in range(2):
                    pre = {(0, 0): (pt0[0], 8), (0, 1): (pt0[1], 8),
                           (1, 0): (pt1h0, PRE_KG)}.get((idx, hp))
                    ptAB, kg_pre = pre if pre else (
                        wpool.tile([128, 2 * KT * q_len], BF16, tag="pt",
                                   bufs=3, name=f"pt_{idx}_{hp}"), 0)
                    oaccs = [pp.tile([VW, 512], F32, tag="oacc", bufs=2,
                                     name=f"oacc_{idx}_{2 * hp + half}")
                             for half in range(2)]
                    def v_group(kg):
                        for half in range(2):
                            for j in range(G):
                                kt = G * kg + j
                                v_mm(oaccs[half], 2 * hp + half, kt, ptAB,
                                     half, kt == 0, kt == KT - 1)
                    # V matmuls trail the scores/exp by one group so the PE
                    # never head-of-line blocks on the exp it just requested
                    for kg in range(KT // G):
                        if kg >= kg_pre:
                            scores_exp(ch, hp, kg, ptAB)
                        if kg == 0:
                            for args in pending_norm:
                                normalize(*args)
                            pending_norm = []
                        else:
                            v_group(kg - 1)
                        if kg == 1:
                            if hp == 0 and pending is not None:
                                # previous chunk: ship its hp=1 heads
                                out_send(pending, 1)
                                sent.append(pending)
                                pending = None
                            elif hp == 1:
                                # this chunk's hp=0 heads are normalized by
                                # now -- ship them early (trigger before
                                # fin's readbacks claim the gpsimd queue)
                                out_send(ch, 0)
                                # project a chunk whose exchange has had a
                                # full chunk of slack (absorbs launch skew
                                # across cores)
                                if len(sent) >= 2:
                                    out_fin(sent.pop(0))
                    v_group(KT // G - 1)
                    for half in range(2):
                        pending_norm.append((ch, hp, half, oaccs[half]))
                pending = ch
            for args in pending_norm:
                normalize(*args)
            out_send(pending, 1)
            sent.append(pending)
            for ch in sent:
                out_fin(ch)

    nc.compile()
    _NC_CACHE[key] = nc
    return nc


def _prep_in_maps(x, qkv_w, qkv_b, out_w, out_b):
    """Per-core input tensors; w2/bo are now the FULL out_w/out_b."""
    mats = _rope_mats()
    x = np.asarray(x, np.float32)
    qkv_w = np.asarray(qkv_w, np.float32)
    qkv_b = np.asarray(qkv_b, np.float32)
    out_w = np.asarray(out_w, np.float32)
    out_b = np.asarray(out_b, np.float32)

    # per-head slices of interleaved qkv (head h owns cols 192h .. 192h+192)
    wq = np.stack([qkv_w[:, 192 * h:192 * h + 64] for h in range(HEADS)])      # [16,1024,64]
    wk = np.stack([qkv_w[:, 192 * h + 64:192 * h + 128] for h in range(HEADS)])
    wv = np.stack([qkv_w[:, 192 * h + 128:192 * h + 192] for h in range(HEADS)])
    bq = np.stack([qkv_b[192 * h:192 * h + 64] for h in range(HEADS)])
    bk = np.stack([qkv_b[192 * h + 64:192 * h + 128] for h in range(HEADS)])
    bvv = np.stack([qkv_b[192 * h + 128:192 * h + 192] for h in range(HEADS)])

    import ml_dtypes
    scale = 1.0 / np.sqrt(D)
    wq_r = np.einsum("hij,hjk->hik", wq, mats) * scale
    bq_r = np.einsum("hj,hjk->hk", bq, mats) * scale
    wk_r = np.einsum("hij,hjk->hik", wk, mats)
    bk_r = np.einsum("hj,hjk->hk", bk, mats)

    in_maps = []
    for c in range(N_CORES):
        g, r = divmod(c, 4)
        hs = [4 * r + i for i in range(HPC)]
        xt = x[g].T.astype(ml_dtypes.bfloat16)                              # [1024, 2048]
        wall_c = np.concatenate([wq_r[h] for h in hs] + [wk_r[h] for h in hs]
                                + [wv[h] for h in hs], axis=1)              # [1024, 768]
        w2_c = out_w                                                        # [1024, 1024]
        ball_c = np.concatenate([bq_r[h] for h in hs] + [bk_r[h] for h in hs]
                                + [bvv[h] for h in hs])                     # [768]
        bo_c = out_b[None, :]
        in_maps.append({
            "xt": xt,
            "wall": wall_c.astype(ml_dtypes.bfloat16),
            "w2": w2_c.astype(ml_dtypes.bfloat16),
            "ball": ball_c.reshape(6, 128).T.copy().astype(np.float32),
            "bo": bo_c.astype(np.float32),
            "ones_i": np.ones((1, 128), np.float32),
            "ident": np.eye(128, dtype=ml_dtypes.bfloat16),
            "vones": np.ones((128, KT * HPC), ml_dtypes.bfloat16),
        })
    return in_maps


RUN_KWARGS = {}     # test.py sets {"trace": True} to profile; harness leaves {}
LAST_RES = None


def kernel(x, qkv_w, qkv_b, out_w, out_b):
    global LAST_RES
    in_maps = _prep_in_maps(x, qkv_w, qkv_b, out_w, out_b)
    nc = _build(with_collectives=True)
    res = None
    for attempt, backoff in enumerate((10, 20, 40, 60, 0)):
        try:
            res = bass_utils.run_bass_kernel_spmd(nc, in_maps,
                                                  core_ids=list(range(N_CORES)),
                                                  **RUN_KWARGS)
            break
        except Exception:
            if backoff == 0:
                raise
            import time as _time
            _time.sleep(backoff)
    LAST_RES = res
    out = np.empty((2, SEQ, HID), np.float32)
    for c in range(N_CORES):
        o = res.results[c]["out"]            # [512, 1024]
        # chunk j rows 128j..128j+128 = [batch0 64 | batch1 64] of q rows
        # [512j + 64c .. +64)
        for j in range(4):
            for b in range(2):
                out[b, 512 * j + 64 * c:512 * j + 64 * (c + 1)] = \
                    o[128 * j + 64 * b:128 * j + 64 * (b + 1)]
    return out



# revision 53
# speedup vs baseline: 1.0172x; 1.0114x over previous
"""Distributed multi-head attention kernel for 8 TRN2 NeuronCores.

Problem: x[2,2048,1024] -> qkv proj -> per-head RoPE (indexed by HEAD, a
fixed linear transform) -> attention (16 heads, d=64) -> out proj.

Sharding: core c handles batch c//4 and heads 4*(c%4) .. 4*(c%4)+3.
After attention, a per-chunk 4-rank AllToAll exchanges bf16 attention
outputs so each core holds ALL 16 heads for its own 128-row q-slice of
the chunk; the out projection then runs locally with the full out_w
(K=1024 accumulated in PSUM) -- no reduction collective anywhere, and
the host only concatenates row blocks.

Host-side folds: RoPE rotation and the 1/sqrt(64) score scale are folded
into qkv_w columns; x and all weights stream as bf16 (projection error
~0.45% rel, well under the 2e-2 gate). P = exp(S) and V are bf16;
softmax denominators come from a ones-column in V_aug.

The exchange is one 8-rank AllToAll per (chunk, head pair): 128KB each,
hp0's launches mid-chunk, hp1's right after the chunk, and the readback
plus out-projection (fin) runs two chunk-phases later so cross-core
launch skew is absorbed by pipeline slack. A tiny warm-up AllToAll in
pass 1 swallows the collective stream's first-op latency and the start
barrier. The two 64-row score matmuls of a head pair write one shared
PSUM tile so the scheduler keeps them adjacent and they run concurrently
in the two halves of the PE array (tile_position row groups). Softmax
1/Z uses reciprocal_approx_fast on a partition-0 staged copy (the custom
DVE op misreads partition-offset inputs). Pass 1 prefills all of chunk
0's and 6/8 of chunk 1-hp0's exp groups into the otherwise idle ACT
engine, which paces pass 2.
"""
import sys
for _p in ("/opt/trn_rl_repo", "/root/.axon_site/_ro/trn_rl_repo"):
    if _p not in sys.path:
        sys.path.insert(0, _p)

import numpy as np

from concourse import bacc, tile, bass_utils
from concourse import mybir

F32 = mybir.dt.float32
F32R = mybir.dt.float32r
BF16 = mybir.dt.bfloat16
F8 = mybir.dt.float8e4
DR = mybir.MatmulPerfMode.DoubleRow
W8 = 32.0           # fp8 weight pre-scale (host); undone via exp scale/vones
EXP = mybir.ActivationFunctionType.Exp

HID = 1024
SEQ = 2048
HEADS = 16
D = 64
HPC = 4            # heads per core
N_CORES = 8
QC = 512           # q-chunk (free dim of scores matmuls)
NQ = SEQ // QC     # 4 q-chunks
KT = SEQ // 128    # 16 key tiles
VW = D + 1         # v_aug width per head (ones column at 64)


def _round_tf32(x):
    u = np.ascontiguousarray(x, dtype=np.float32).view(np.uint32).copy()
    u += 0xFFF + ((u >> 13) & 1)
    u &= np.uint32(0xFFFFE000)
    return u.view(np.float32)


def _rope_mats():
    """M_h [64,64] per head h: q_rot = q @ M_h (head-indexed RoPE quirk)."""
    j = np.arange(0, D, 2, dtype=np.float64) / D
    inv_freq = 1.0 / (10000.0 ** j)              # [32]
    h = np.arange(HEADS, dtype=np.float64)
    freqs = h[:, None] * inv_freq[None, :]       # [16, 32]
    cos = np.cos(freqs).astype(np.float32)
    sin = np.sin(freqs).astype(np.float32)
    mats = np.zeros((HEADS, D, D), np.float32)
    idx = np.arange(D // 2)
    for hh in range(HEADS):
        mats[hh, idx, idx] = cos[hh]
        mats[hh, D // 2 + idx, idx] = -sin[hh]
        mats[hh, idx, D // 2 + idx] = sin[hh]
        mats[hh, D // 2 + idx, D // 2 + idx] = cos[hh]
    return mats


_NC_CACHE = {}


def _build(with_collectives=True, n_cores=N_CORES):
    key = (with_collectives, n_cores)
    if key in _NC_CACHE:
        return _NC_CACHE[key]
    nc = bacc.Bacc("TRN2", target_bir_lowering=False, debug=False,
                   num_devices=n_cores)

    # weight column tiles ct: 0=q01 1=q23 2=k01 3=k23 4=v01 5=v23
    xt = nc.dram_tensor("xt", [HID, SEQ], BF16, kind="ExternalInput")
    wall = nc.dram_tensor("wall", [HID, 12 * D], BF16, kind="ExternalInput")
    w2 = nc.dram_tensor("w2", [HID, HID], BF16, kind="ExternalInput")
    ball = nc.dram_tensor("ball", [128, 6], F32, kind="ExternalInput")
    bo = nc.dram_tensor("bo", [1, HID], F32, kind="ExternalInput")
    ones_i = nc.dram_tensor("ones_i", [1, 128], F32, kind="ExternalInput")
    ident = nc.dram_tensor("ident", [128, 128], BF16, kind="ExternalInput")
    vones = nc.dram_tensor("vones", [128, KT * HPC], BF16, kind="ExternalInput")
    out_e = nc.dram_tensor("out", [QC, HID], F32, kind="ExternalOutput")

    with tile.TileContext(nc) as tc:
        with tc.tile_pool(name="const", bufs=1) as cpool, \
             tc.tile_pool(name="work", bufs=1) as wpool, \
             tc.tile_pool(name="xts", bufs=1) as xpool, \
             tc.tile_pool(name="psum", bufs=1, space="PSUM") as pp, \
             tc.tile_pool(name="dram", bufs=1, space="DRAM") as dpool:

            # ---- constant loads
            wall_sb = cpool.tile([128, 8 * 768], BF16)     # k-tile k at [:, 768k:+768]
            w2_sb = cpool.tile([128, 8 * HID], BF16)       # full out_w, row-block kb at [:, kb*HID:+HID]
            ball_sb = cpool.tile([128, 6], F32)
            bo_sb = cpool.tile([1, HID], F32)
            ones_sb = cpool.tile([1, 128], F32)
            id_sb = cpool.tile([128, 128], BF16)
            nc.gpsimd.dma_start(id_sb[:], ident.ap()[:])
            nc.gpsimd.dma_start(ball_sb[:], ball.ap()[:])
            bob_sb = cpool.tile([128, HID], F32)

            # ---- persistent activations
            qkT_sb = wpool.tile([128, 4 * SEQ], BF16)   # col-tile ct at [:, ct*SEQ:+SEQ]
            vT_sb = wpool.tile([128, 2 * SEQ], BF16)
            v_sb = wpool.tile([128, KT * HPC * VW], BF16)
            outT_sb = wpool.tile([128, 2 * SEQ], BF16)

            def xt_dma(nq, k, eng):
                t = xpool.tile([128, 512], BF16, tag="xts", bufs=10,
                               name=f"xt_{nq}_{k}")
                eng.dma_start(t[:], xt.ap()[128 * k:128 * (k + 1),
                                            QC * nq:QC * (nq + 1)])
                return t

            CH0 = (0, 0, 512)
            CH1 = (1, 512, 512)
            # P blocks filled during pass 1: all of q-chunk 0 (one tile per
            # head pair) plus chunk 1 hp0's first 4 groups; key tile kt's
            # [halfA | halfB] block lives at [:, 1024*kt:+1024]
            pt0 = [wpool.tile([128, 2 * KT * QC], BF16, tag="pt", bufs=3,
                              name=f"pt0_{i}") for i in range(2)]
            pt1h0 = wpool.tile([128, 2 * KT * QC], BF16, tag="pt", bufs=3,
                               name="pt1h0")
            PRE_KG = 6         # chunk-1 hp0 groups prefetched in pass 1

            def scores_exp(ch, hp, kg, pt):
                """S^T matmuls for head pair hp of one exp-group: 2 key
                tiles of q-chunk ch; exp directly PSUM -> SBUF (bf16).
                Both 64-row halves write ONE ps tile so they become ready
                together and overlap in the two array halves."""
                if pt is None:
                    return
                _, q_off, q_len = ch
                assert q_len == QC
                qslc = slice(SEQ * hp + q_off, SEQ * hp + q_off + q_len)
                for j in range(2):
                    kt = 2 * kg + j
                    kslc = slice(SEQ * (2 + hp) + 128 * kt,
                                 SEQ * (2 + hp) + 128 * (kt + 1))
                    ps = pp.tile([128, 1024], F32, tag="s", bufs=2,
                                 name=f"ps_{q_off}_{hp}_{kt}")
                    for base in (0, 64):
                        nc.tensor.matmul(ps[:, 8 * base:8 * base + 512],
                                         lhsT=qkT_sb[base:base + 64, kslc],
                                         rhs=qkT_sb[base:base + 64, qslc],
                                         start=True, stop=True,
                                         tile_position=(base, 0))
                    nc.scalar.activation(pt[:, 1024 * kt:1024 * (kt + 1)],
                                         ps[:], EXP)

            def v_mm(oacc, h, kt, pt, half, start, stop):
                nc.tensor.matmul(
                    oacc[:, :QC],
                    lhsT=v_sb[:, VW * HPC * kt + VW * h:VW * HPC * kt + VW * (h + 1)],
                    rhs=pt[:, 1024 * kt + 512 * half:1024 * kt + 512 * (half + 1)],
                    start=start, stop=stop)

            def normalize(ch, hp, half, oacc):
                _, q_off, q_len = ch
                h = 2 * hp + half
                # stage Z at partition 0: reciprocal_approx_fast reads
                # partition-offset inputs incorrectly (measured), and ~5x
                # faster than exact reciprocal otherwise
                zs = wpool.tile([1, 512], F32, tag="zs", bufs=2,
                                name=f"zs_{q_off}_{h}")
                nc.vector.tensor_copy(zs[:, :q_len], oacc[D:D + 1, :q_len])
                rz = wpool.tile([1, 512], F32, tag="rz", bufs=2,
                                name=f"rz_{q_off}_{h}")
                with nc.allow_low_precision(reason="recip feeds bcast matmul"):
                    nc.vector.reciprocal_approx_fast(rz[:, :q_len],
                                                     zs[:, :q_len])
                # copy the unnormalized head output out of PSUM while the
                # reciprocal/broadcast run; the multiply then reads the
                # broadcast straight from PSUM (one DVE hop shorter)
                ou = wpool.tile([64, 512], F32, tag="bc", bufs=2, name=f"ou_{q_off}_{h}")
                nc.vector.tensor_copy(ou[:, :q_len], oacc[0:D, :q_len])
                bcm = pp.tile([64, 512], F32, tag="pr", bufs=2, name=f"bcm_{q_off}_{h}")
                nc.tensor.matmul(bcm[:, :q_len], lhsT=ones_sb[:, :64],
                                 rhs=rz[:, :q_len], start=True, stop=True)
                nc.vector.tensor_tensor(
                    outT_sb[64 * half:64 * (half + 1),
                            SEQ * hp + q_off:SEQ * hp + q_off + q_len],
                    bcm[:, :q_len], ou[:, :q_len],
                    mybir.AluOpType.mult)

            # ~5us of dummy matmuls at t=0: releases the HAM clock gate
            # before the real stream starts (the PE otherwise idles waiting
            # for the first x tiles and begins pass 1 cold at 1.2 GHz)
            spin = pp.tile([128, 128], F32, tag="pr", bufs=2, name="spin")
            for _ in range(24):
                nc.tensor.matmul(spin[:], lhsT=id_sb[:], rhs=id_sb[:],
                                 start=True, stop=True)

            # ---- pass 1: project K^T, then Q^T, then V^T chunk by chunk
            # (3 sweeps over resident x^T tiles); early scores+exp for
            # q-chunk 0 keep ACT busy while the PE projects.
            for nq in range(NQ):
                if nq == 1 and with_collectives:
                    # warm-up AllToAll (16KB of identity-matrix bytes):
                    # absorbs the collective stream's first-op latency (and
                    # the start barrier) mid pass 1 so chunk 0's real
                    # exchange starts promptly.
                    warm_in = dpool.tile([8 * 128, 8], BF16, name="warm_in")
                    warm_out = dpool.tile([8 * 128, 8], BF16, name="warm_out")
                    for a in range(8):
                        nc.gpsimd.dma_start(warm_in[128 * a:128 * (a + 1), :],
                                            id_sb[:, 0:8])
                    nc.gpsimd.collective_compute(
                        "AllToAll",
                        mybir.AluOpType.bypass,
                        replica_groups=[list(range(8))],
                        ins=[warm_in[:].opt()],
                        outs=[warm_out[:].opt()],
                    )
                # per-column-tile 1-bank accumulators (tags oacc/pr) keep
                # the whole "s" tag free for score tiles -- pass-1 proj no
                # longer serializes against the early-chunk exps
                def proj_pair(cts, xts_seq, tag_pair=("oacc", "pr")):
                    accs = [pp.tile([128, 512], F32, tag=tg, bufs=2,
                                    name=f"acc{ct}_{nq}")
                            for ct, tg in zip(cts, tag_pair)]
                    for k, xt_t in xts_seq:
                        for acc, ct in zip(accs, cts):
                            nc.tensor.matmul(
                                acc[:],
                                lhsT=wall_sb[:, 768 * k + 128 * ct:768 * k + 128 * (ct + 1)],
                                rhs=xt_t[:], start=(k == 0), stop=(k == 7))
                    for acc, ct in zip(accs, cts):
                        nc.vector.tensor_scalar_add(
                            qkT_sb[:, SEQ * ct + QC * nq:SEQ * ct + QC * (nq + 1)],
                            acc[:], ball_sb[:, ct:ct + 1])

                xts = []

                def xts_stream():
                    for k in range(8):
                        if nq == 0:
                            nc.gpsimd.dma_start(
                                wall_sb[:, 768 * k:768 * (k + 1)],
                                wall.ap()[128 * k:128 * (k + 1), :])
                        # spread the x^T stream over 3 DMA queues (gpsimd
                        # is loading weights during nq 0)
                        if nq == 0:
                            xt_eng = nc.sync if k % 2 == 0 else nc.scalar
                        else:
                            xt_eng = (nc.sync, nc.scalar, nc.gpsimd)[k % 3]
                        xt_t = xt_dma(nq, k, xt_eng)
                        xts.append(xt_t)
                        yield k, xt_t

                if nq == 0:
                    # chunk 0: K and Q interleaved so the first scores fire
                    # asap (4 live accumulator banks)
                    stream = xts_stream()
                    accs = [pp.tile([128, 512], F32, tag=tg, bufs=2,
                                    name=f"acc{ct}_0")
                            for ct, tg in zip((2, 3, 0, 1),
                                              ("oacc", "pr", "oacc", "pr"))]
                    for k, xt_t in stream:
                        for acc, ct in zip(accs, (2, 3, 0, 1)):
                            nc.tensor.matmul(
                                acc[:],
                                lhsT=wall_sb[:, 768 * k + 128 * ct:768 * k + 128 * (ct + 1)],
                                rhs=xt_t[:], start=(k == 0), stop=(k == 7))
                    for acc, ct in zip(accs, (2, 3, 0, 1)):
                        nc.vector.tensor_scalar_add(
                            qkT_sb[:, SEQ * ct + QC * nq:SEQ * ct + QC * (nq + 1)],
                            acc[:], ball_sb[:, ct:ct + 1])
                    scores_exp(CH0, 0, 0, pt0[0])
                else:
                    proj_pair((2, 3), xts_stream())
                    scores_exp(CH0, 0, 2 * nq, pt0[0])
                    proj_pair((0, 1), ((k, xts[k]) for k in range(8)))
                scores_exp(CH0, 1, 2 * nq, pt0[1])
                scores_exp(CH0, 0, 2 * nq + 1, pt0[0])
                if nq >= 2:
                    # chunk-1 hp0 prefetch: Q(ch1) ready after nq=1, K key
                    # tiles 0..11 after nq=2 -- steal idle ACT time here
                    n_pre, base_kg = (2, 0) if nq == 2 else (4, 2)
                    for kgp in range(base_kg, base_kg + n_pre):
                        scores_exp(CH1, 0, kgp, pt1h0)
                vA = pp.tile([128, 512], F32, tag="oacc", bufs=2, name=f"vA_{nq}")
                vB = pp.tile([128, 512], F32, tag="pr", bufs=2, name=f"vB_{nq}")
                for k in range(8):
                    nc.tensor.matmul(
                        vA[:], lhsT=wall_sb[:, 768 * k + 512:768 * k + 640],
                        rhs=xts[k][:], start=(k == 0), stop=(k == 7))
                    nc.tensor.matmul(
                        vB[:], lhsT=wall_sb[:, 768 * k + 640:768 * k + 768],
                        rhs=xts[k][:], start=(k == 0), stop=(k == 7))
                nc.vector.tensor_scalar_add(
                    vT_sb[:, QC * nq:QC * (nq + 1)], vA[:], ball_sb[:, 4:5])
                nc.vector.tensor_scalar_add(
                    vT_sb[:, SEQ + QC * nq:SEQ + QC * (nq + 1)], vB[:], ball_sb[:, 5:6])
                # V^T -> V (natural, bf16) for this quarter of the keys
                for cv in range(2):
                    for st in range(4 * nq, 4 * nq + 4):
                        tp = pp.tile([128, 128], BF16, tag="pr", bufs=2,
                                     name=f"tp_{cv}_{st}")
                        nc.tensor.transpose(
                            tp[:], vT_sb[:, SEQ * cv + 128 * st:SEQ * cv + 128 * (st + 1)],
                            id_sb[:])
                        dst = v_sb[:, VW * HPC * st + 2 * VW * cv:VW * HPC * st + 2 * VW * (cv + 1)]
                        nc.vector.tensor_copy(
                            dst.rearrange("p (h w) -> p h w", h=2, w=VW)[:, :, :D],
                            tp[:].rearrange("p (h w) -> p h w", h=2, w=D),
                        )
                scores_exp(CH0, 1, 2 * nq + 1, pt0[1])

            # ones columns of v_aug: one strided DMA ([128, 64] with free stride VW)
            nc.sync.dma_start(
                v_sb[:].rearrange("p (i w) -> p i w", i=KT * HPC, w=VW)[:, :, D],
                vones.ap()[:],
            )
            # ---- deferred constants (needed from pass 2 on)
            for k in range(8):
                nc.sync.dma_start(w2_sb[:, HID * k:HID * (k + 1)],
                                  w2.ap()[128 * k:128 * (k + 1), :])
            nc.sync.dma_start(bo_sb[:], bo.ap()[:])
            nc.sync.dma_start(ones_sb[:], ones_i.ap()[:])
            # out-bias broadcast ([1,N] -> [128,N] via K=1 ones matmul)
            for nn in range(2):
                ps_bo = pp.tile([128, 512], F32, tag="pr", bufs=2)
                nc.tensor.matmul(ps_bo[:], lhsT=ones_sb[:, :128],
                                 rhs=bo_sb[:, 512 * nn:512 * (nn + 1)],
                                 start=True, stop=True)
                nc.vector.tensor_copy(bob_sb[:, 512 * nn:512 * (nn + 1)], ps_bo[:])

            # q-chunks: (index, q_off, q_len)
            CHUNKS = [(0, 0, 512), (1, 512, 512), (2, 1024, 512),
                      (3, 1536, 512)]
            # per-chunk output row offset in out_e
            OUT_ROW = {0: 0, 1: 128, 2: 256, 3: 384}
            QPB = 64           # q rows per batch per core per chunk

            # ---- pass 2 out projection, 8-rank AllToAll per (chunk, head
            # pair): dst core c' gets this core's heads 2hp,2hp+1 for q rows
            # [q_off+64c' .. +64); afterwards each core holds ALL 16 heads of
            # BOTH batches for its own 64-row q-slice and runs the out
            # projection locally (batches packed side by side, K=1024).
            # hp=0's exchange launches mid-chunk, hp=1's right after the
            # chunk; the PE-consuming fin runs deep inside the next chunk.
            a2a_outs = {}

            def out_send(ch, hp):
                idx, q_off, q_len = ch
                a2a_in = dpool.tile([8 * 128, QPB], BF16, tag="a2ain",
                                    bufs=6, name=f"a2ain_{idx}_{hp}")
                for cd in range(8):
                    nc.sync.dma_start(
                        a2a_in[128 * cd:128 * (cd + 1), :],
                        outT_sb[:, SEQ * hp + q_off + QPB * cd:
                                SEQ * hp + q_off + QPB * (cd + 1)])
                a2a_out = dpool.tile([8 * 128, QPB], BF16, tag="a2aout",
                                     bufs=6, name=f"a2aout_{idx}_{hp}")
                a2a_outs[(idx, hp)] = a2a_out
                if with_collectives:
                    nc.gpsimd.collective_compute(
                        "AllToAll",
                        mybir.AluOpType.bypass,
                        replica_groups=[list(range(8))],
                        ins=[a2a_in[:].opt()],
                        outs=[a2a_out[:].opt()],
                    )
                else:
                    nc.sync.dma_start(a2a_out[:], a2a_in[:])

            def out_fin(ch):
                idx, q_off, q_len = ch
                # agg block kb=2*rs+hp holds out_w rows 128kb..128(kb+1) as
                # partitions; cols 0:64 = batch 0 (core rs), 64:128 = batch 1
                # (core rs+4), q rows are this core's own slice.
                agg = wpool.tile([128, 8 * 128], BF16, tag="agg", bufs=2,
                                 name=f"agg_{idx}")
                for hp in range(2):
                    a2a_out = a2a_outs.pop((idx, hp))
                    for s in range(8):
                        rs, gs = s % 4, s // 4
                        kb = 2 * rs + hp
                        nc.gpsimd.dma_start(
                            agg[:, 128 * kb + QPB * gs:128 * kb + QPB * (gs + 1)],
                            a2a_out[128 * s:128 * (s + 1), :])
                ob = wpool.tile([128, HID], F32, tag="ob", bufs=2,
                                name=f"ob_{idx}")
                psos = [pp.tile([128, 512], F32, tag="pr", bufs=2,
                                name=f"pso_{idx}_{nn}") for nn in range(2)]
                for kb in range(8):
                    for nn in range(2):
                        nc.tensor.matmul(
                            psos[nn][:],
                            lhsT=agg[:, 128 * kb:128 * (kb + 1)],
                            rhs=w2_sb[:, HID * kb + 512 * nn:HID * kb + 512 * (nn + 1)],
                            start=(kb == 0), stop=(kb == 7))
                for nn in range(2):
                    nc.vector.tensor_tensor(
                        ob[:, 512 * nn:512 * (nn + 1)], psos[nn][:],
                        bob_sb[:, 512 * nn:512 * (nn + 1)],
                        mybir.AluOpType.add)
                nc.sync.dma_start(
                    out_e.ap()[OUT_ROW[idx]:OUT_ROW[idx] + 128, :], ob[:, :HID])

            sent = []          # chunks whose hp=1 exchange is in flight
            pending = None
            pending_norm = []
            for ch in CHUNKS:
                idx, q_off, q_len = ch
                G = 1024 // q_len
                for hp in range(2):
                    pre = {(0, 0): (pt0[0], 8), (0, 1): (pt0[1], 8),
                           (1, 0): (pt1h0, PRE_KG)}.get((idx, hp))
                    ptAB, kg_pre = pre if pre else (
                        wpool.tile([128, 2 * KT * q_len], BF16, tag="pt",
                                   bufs=3, name=f"pt_{idx}_{hp}"), 0)
                    oaccs = [pp.tile([VW, 512], F32, tag="oacc", bufs=2,
                                     name=f"oacc_{idx}_{2 * hp + half}")
                             for half in range(2)]
                    def v_group(kg):
                        for half in range(2):
                            for j in range(G):
                                kt = G * kg + j
                                v_mm(oaccs[half], 2 * hp + half, kt, ptAB,
                                     half, kt == 0, kt == KT - 1)
                    # V matmuls trail the scores/exp by one group so the PE
                    # never head-of-line blocks on the exp it just requested
                    for kg in range(KT // G):
                        if kg >= kg_pre:
                            scores_exp(ch, hp, kg, ptAB)
                        if kg == 0:
                            for args in pending_norm:
                                normalize(*args)
                            pending_norm = []
                        else:
                            v_group(kg - 1)
                        if kg == 1:
                            if hp == 0 and pending is not None:
                                # previous chunk: ship its hp=1 heads
                                out_send(pending, 1)
                                sent.append(pending)
                                pending = None
                            elif hp == 1:
                                # this chunk's hp=0 heads are normalized by
                                # now -- ship them early (trigger before
                                # fin's readbacks claim the gpsimd queue)
                                out_send(ch, 0)
                                # project a chunk whose exchange has had a
                                # full chunk of slack (absorbs launch skew
                                # across cores)
                                if len(sent) >= 2:
                                    out_fin(sent.pop(0))
                    v_group(KT // G - 1)
                    for half in range(2):
                        pending_norm.append((ch, hp, half, oaccs[half]))
                pending = ch
            for args in pending_norm:
                normalize(*args)
            out_send(pending, 1)
            sent.append(pending)
            for ch in sent:
                out_fin(ch)

    nc.compile()
    _NC_CACHE[key] = nc
    return nc


def _prep_in_maps(x, qkv_w, qkv_b, out_w, out_b):
    """Per-core input tensors; w2/bo are now the FULL out_w/out_b."""
    mats = _rope_mats()
    x = np.asarray(x, np.float32)
    qkv_w = np.asarray(qkv_w, np.float32)
    qkv_b = np.asarray(qkv_b, np.float32)
    out_w = np.asarray(out_w, np.float32)
    out_b = np.asarray(out_b, np.float32)

    # per-head slices of interleaved qkv (head h owns cols 192h .. 192h+192)
    wq = np.stack([qkv_w[:, 192 * h:192 * h + 64] for h in range(HEADS)])      # [16,1024,64]
    wk = np.stack([qkv_w[:, 192 * h + 64:192 * h + 128] for h in range(HEADS)])
    wv = np.stack([qkv_w[:, 192 * h + 128:192 * h + 192] for h in range(HEADS)])
    bq = np.stack([qkv_b[192 * h:192 * h + 64] for h in range(HEADS)])
    bk = np.stack([qkv_b[192 * h + 64:192 * h + 128] for h in range(HEADS)])
    bvv = np.stack([qkv_b[192 * h + 128:192 * h + 192] for h in range(HEADS)])

    import ml_dtypes
    scale = 1.0 / np.sqrt(D)
    wq_r = np.einsum("hij,hjk->hik", wq, mats) * scale
    bq_r = np.einsum("hj,hjk->hk", bq, mats) * scale
    wk_r = np.einsum("hij,hjk->hik", wk, mats)
    bk_r = np.einsum("hj,hjk->hk", bk, mats)

    in_maps = []
    for c in range(N_CORES):
        g, r = divmod(c, 4)
        hs = [4 * r + i for i in range(HPC)]
        xt = x[g].T.astype(ml_dtypes.bfloat16)                              # [1024, 2048]
        wall_c = np.concatenate([wq_r[h] for h in hs] + [wk_r[h] for h in hs]
                                + [wv[h] for h in hs], axis=1)              # [1024, 768]
        w2_c = out_w                                                        # [1024, 1024]
        ball_c = np.concatenate([bq_r[h] for h in hs] + [bk_r[h] for h in hs]
                                + [bvv[h] for h in hs])                     # [768]
        bo_c = out_b[None, :]
        in_maps.append({
            "xt": xt,
            "wall": wall_c.astype(ml_dtypes.bfloat16),
            "w2": w2_c.astype(ml_dtypes.bfloat16),
            "ball": ball_c.reshape(6, 128).T.copy().astype(np.float32),
            "bo": bo_c.astype(np.float32),
            "ones_i": np.ones((1, 128), np.float32),
            "ident": np.eye(128, dtype=ml_dtypes.bfloat16),
            "vones": np.ones((128, KT * HPC), ml_dtypes.bfloat16),
        })
    return in_maps


RUN_KWARGS = {}     # test.py sets {"trace": True} to profile; harness leaves {}
LAST_RES = None


def kernel(x, qkv_w, qkv_b, out_w, out_b):
    global LAST_RES
    in_maps = _prep_in_maps(x, qkv_w, qkv_b, out_w, out_b)
    nc = _build(with_collectives=True)
    res = None
    for attempt, backoff in enumerate((10, 20, 40, 60, 0)):
        try:
            res = bass_utils.run_bass_kernel_spmd(nc, in_maps,
                                                  core_ids=list(range(N_CORES)),
                                                  **RUN_KWARGS)
            break
        except Exception:
            if backoff == 0:
                raise
            import time as _time
            _time.sleep(backoff)
    LAST_RES = res
    out = np.empty((2, SEQ, HID), np.float32)
    for c in range(N_CORES):
        o = res.results[c]["out"]            # [512, 1024]
        # chunk j rows 128j..128j+128 = [batch0 64 | batch1 64] of q rows
        # [512j + 64c .. +64)
        for j in range(4):
            for b in range(2):
                out[b, 512 * j + 64 * c:512 * j + 64 * (c + 1)] = \
                    o[128 * j + 64 * b:128 * j + 64 * (b + 1)]
    return out

